# revision 15
# baseline (speedup 1.0000x reference)
"""Trainium2 Bass kernel for nn_AggregationLayer2 (5x5 spatially-varying
neighborhood aggregation, 26 slots: 25 spatial shifts + current value).

    out[b,h,w,c] = sum_k attn[b,h,w,k] * neighbor_k(ref_value)[c]
                 + attn[b,h,w,25] * current_ref_value[b,h,w,c]

Strategy (8 NeuronCores, SPMD), v3:
  - Shard: (batch, H-half) -> 8 shards of 64 output rows each; host ships
    ref rows with a 2-row zero halo, bf16.
  - Compute: per output row h and vertical offset b, the dj-contraction is
    a banded matmul out_row[w,c] += sum_{w'} BandT[w',w] * ref[h+b,w',c];
    5 PSUM-accumulated matmuls per output row on the TensorEngine.
  - Band storage: groups of R=8 rows interleaved so that element
    (col c, band b, row rr) sits at c*5R + b*R + rr. The PE reads a clean
    [128 x 128] AP at stride 5R; each partition w' holds its 25*R values
    in ONE contiguous 400B run at offset (w'+14)*5R -> the attn stream is
    shipped compact (0.4MB vs 2.9MB zero-inflated) with 128 descriptors
    per group instead of 25x that.
  - Zeros: band ring of 3 group-slots; gaps are memset once at startup
    (split across DVE/Act/Pool) and never dirtied - each reuse rewrites
    exactly the same value cells.
  - Output in bf16 (halves the output DMA); host upcasts.
  - Current term attn[...,25]*current is pre-scaled on the host (fp32) and
    folded in during 2-row PSUM evictions alternating DVE/Act.
  - PE warmup matmuls on scratch data raise the PE p-state during the
    input-DMA window so real matmuls run at full clock.
"""

import numpy as np
import ml_dtypes

import concourse.bass as bass
import concourse.mybir as mybir
from concourse.tile import TileContext
from concourse.tile_rust import add_dep_helper
from concourse.vector_clock import ScopedClock
from concourse import bass_utils

# ---------------------------------------------------------------------------
# Toolchain compat: this walrus build codegens at most one sync-wait command
# per instruction and rejects eq-mode waits on Drain ops. Replace the Tile
# tail barrier and split multi-waits onto standalone EventSemaphore waits.
# ---------------------------------------------------------------------------

_wsplit_counter = [0]


def _split_multi_waits(nc):
    for f in nc.m.functions:
        for bb in f.blocks:
            out = []
            changed = False
            for inst in bb.instructions:
                si = inst.sync_info
                if si is not None and len(si.on_wait) > 1:
                    waits = list(si.on_wait)
                    for w in waits[:-1]:
                        _wsplit_counter[0] += 1
                        ev = mybir.InstEventSemaphore(
                            name=f"WSPLIT-{_wsplit_counter[0]}",
                            engine=inst.engine,
                            ins=[],
                            outs=[],
                            sync_info=mybir.SyncInfo(on_wait=[w], on_update=[]),
                        )
                        out.append(ev)
                    si.on_wait = [waits[-1]]
                    changed = True
                out.append(inst)
            if changed:
                bb.instructions = out


def _drain_and_barrier_compat(self, tick_clock, wait_clock):
    nc = self.nc
    carrier = nc.sync.nop()
    wait_clock.add_sem_waits(
        carrier.ins, ScopedClock({None: tick_clock.global_clock})
    )
    waits = list(carrier.ins.sync_info.on_wait)
    if len(waits) > 1:
        carrier.ins.sync_info.on_wait = [waits[0]]
        engines = list(nc.engines.values())
        for idx, w in enumerate(waits[1:]):
            n = engines[idx % len(engines)].nop()
            n.ins.sync_info = mybir.SyncInfo(on_wait=[w], on_update=[])

    barrier_sem = nc.alloc_semaphore("tile_final_barrier")
    n_eng = len(nc.engines)
    for eng in nc.engines.values():
        eng.drain(fusable=False)
        eng.sem_inc(barrier_sem, 1)
        eng.wait_ge(barrier_sem, n_eng)
    for _ in range(4):
        nc.gpsimd.nop()
    nc.gpsimd.sem_clear(barrier_sem)

    popped = nc._tile_sem_poison_stack.pop()
    assert popped is self._sem_poison
    nc.clear_and_free_semaphores(list(self.sems.allocated().values()))


_orig_tc_exit = TileContext.__exit__


def _patched_tc_exit(self, exc_type, exc_value, traceback):
    r = _orig_tc_exit(self, exc_type, exc_value, traceback)
    if not exc_type:
        _split_multi_waits(self.nc)
    return r


def _install_tilefix():
    TileContext._drain_and_barrier = _drain_and_barrier_compat
    TileContext.__exit__ = _patched_tc_exit


_install_tilefix()


def _install_ntff_hook():
    """The image's antenv lacks axon_hooks; provide it and register the
    ctypes NTFF profiling hook so BASS_TRACE=1 yields HW exec times."""
    import sys
    import types

    if "antenv.axon_hooks" not in sys.modules:
        mod = types.ModuleType("antenv.axon_hooks")
        holder = [None]
        mod.set_axon_ntff_profile_hook = lambda h: holder.__setitem__(0, h)
        mod.get_axon_ntff_profile_hook = lambda: holder[0]
        sys.modules["antenv.axon_hooks"] = mod
        try:
            import antenv

            antenv.axon_hooks = mod
        except ImportError:
            pass
    try:
        from antenv.axon_hooks import (
            get_axon_ntff_profile_hook,
            set_axon_ntff_profile_hook,
        )

        if get_axon_ntff_profile_hook() is None:
            from trn_agent_boot.trn_boot import _ntff_profile_via_ctypes

            set_axon_ntff_profile_hook(
                _ntff_profile_via_ctypes("/opt/axon/libaxon_pjrt.so")
            )
    except Exception:
        pass

    # artifact upload needs external storage; degrade to local-only
    def _no_upload(tmpdir):
        return tmpdir

    bass_utils.upload_artifacts = _no_upload


_install_ntff_hook()

# ---------------------------------------------------------------------------
# Problem geometry (hardcoded per the harness contract)
# ---------------------------------------------------------------------------

B, H, W, C = 4, 128, 128, 64
NCORES = 8
HS = H // 2          # 64 output rows per shard
HALO_R = HS + 4      # 68 ref rows incl 2-row halo
RCH = 16             # rows per chunk
NCH = HS // RCH      # 4 chunks
RING = 2             # band ring slots
NSUB = 8             # 2-row subregions per chunk
SRW = 10             # col stride elems (5 bands x 2 rows)
SUBR = 160 * SRW     # 1600: per-partition elems per subregion
FGC = NSUB * SUBR    # 12800: per-partition elems per chunk region
WINS = 36 * SRW      # 360: window elems per (partition, subregion)
NBAND = 5
WARMN = 6            # PE warmup matmuls (N=512 each)

BF16 = mybir.dt.bfloat16
F32 = mybir.dt.float32
U32 = mybir.dt.uint32

bfloat16 = ml_dtypes.bfloat16

# ref piece row ranges (halo rows); chunk c needs rows <= c*RCH + RCH + 4
REF_PIECES = [(0, 22), (22, 38), (38, 54), (54, 68)]


def _build_bass():
    nc = bass.Bass()
    refhl = nc.dram_tensor("refhl", [W, HALO_R, C], BF16, kind="ExternalInput")
    curhl = nc.dram_tensor("curhl", [W, HS, C], BF16, kind="ExternalInput")
    qb = nc.dram_tensor("qb", [NCH * W * NSUB * WINS], BF16, kind="ExternalInput")
    out = nc.dram_tensor("out", [W, HS, C], BF16, kind="ExternalOutput")

    with TileContext(nc) as tc:
        with (
            tc.tile_pool(name="sb", bufs=1) as pool,
            tc.tile_pool(name="band", bufs=RING) as band_pool,
            tc.tile_pool(name="ps", bufs=7, space="PSUM") as psum_pool,
            tc.tile_pool(name="wp", bufs=1, space="PSUM") as warm_pool,
        ):
            refsb = pool.tile([W, HALO_R * C], BF16, tag="refsb")
            cursb = pool.tile([W, HS * C], BF16, tag="cursb")
            outst = pool.tile([W, HS * C], BF16, tag="outst")
            scratch = pool.tile([W, 512], BF16, tag="scratch")

            # --- PE warmup: raise p-state while inputs stream in ---
            nc.vector.memset(scratch[:].bitcast(U32), 0)
            warm_ps = warm_pool.tile([W, 512], F32, tag="warm")
            for _ in range(WARMN):
                nc.tensor.matmul(
                    warm_ps[:],
                    scratch[:, 0:128],
                    scratch[:],
                    start=True,
                    stop=True,
                )

            refv = refhl[:].rearrange("w r c -> w (r c)")
            curv = curhl[:].rearrange("w r c -> w (r c)")

            # chunk-0/1 inputs issue first (Act); the later ref/cur pieces
            # go out on the Pool SWDGE queue after its memsets so the DMA
            # engines stream chunk-0 data ahead of them
            nc.scalar.dma_start(
                out=refsb[:, 0 : 22 * C], in_=refv[:, 0 : 22 * C]
            )
            nc.scalar.dma_start(
                out=cursb[:, 0 : 32 * C], in_=curv[:, 0 : 32 * C]
            )
            a, b = REF_PIECES[1]
            nc.scalar.dma_start(
                out=refsb[:, a * C : b * C], in_=refv[:, a * C : b * C]
            )

            # --- band ring: allocate the first RING generations up front so
            # their zero-memsets run first on DVE/Pool; later chunks rotate
            # through the pool (acquires add the write-after-read deps).
            # Only the PE-read col range [16,144) of each subregion needs
            # zeros; window cells are rewritten by every scatter.
            ring = []
            for s in range(RING):
                t = band_pool.tile([W, FGC], BF16, tag="band")
                u = t[:].rearrange("p (s x) -> p s x", s=NSUB).bitcast(U32)
                half = NSUB // 2
                nc.vector.memset(u[:, 0:half, 80:720], 0)
                nc.gpsimd.memset(u[:, half:, 80:720], 0)
                ring.append(t)

            for ci in range(NCH):
                slot = ring[ci] if ci < RING else band_pool.tile(
                    [W, FGC], BF16, tag="band"
                )
                for g in range(4):
                    c0 = (32 * g + 14) * SRW
                    dst = slot[32 * g : 32 * g + 32, :].rearrange(
                        "p (s x) -> p s x", s=NSUB
                    )[:, :, c0 : c0 + WINS]
                    base = (ci * 4 + g) * 32 * NSUB * WINS
                    src = qb[base : base + 32 * NSUB * WINS].rearrange(
                        "(p s j) -> p s j", p=32, j=WINS
                    )
                    eng = nc.sync if g % 2 == 0 else nc.scalar
                    eng.dma_start(out=dst, in_=src)
                if ci == 0:
                    # later ref/cur pieces, behind chunk-0/1 traffic
                    for i in (2, 3):
                        a, b = REF_PIECES[i]
                        nc.gpsimd.dma_start(
                            out=refsb[:, a * C : b * C],
                            in_=refv[:, a * C : b * C],
                        )
                    nc.gpsimd.dma_start(
                        out=cursb[:, 32 * C :], in_=curv[:, 32 * C :]
                    )

                view = slot[:].rearrange(
                    "p (s c b r) -> p s c b r", s=NSUB, b=NBAND, r=2
                )
                for rp in range(RCH // 2):
                    ps = psum_pool.tile([W, 2 * C], F32, tag="ps")
                    for sub in range(2):
                        rr = 2 * rp + sub
                        h = ci * RCH + rr
                        for b in range(NBAND):
                            lhsT = view[:, rr // 2, 16:144, b, rr % 2]
                            rhs = refsb[:, (h + b) * C : (h + b + 1) * C]
                            nc.tensor.matmul(
                                ps[:, sub * C : (sub + 1) * C],
                                lhsT,
                                rhs,
                                start=(b == 0),
                                stop=(b == NBAND - 1),
                            )
                    he = ci * RCH + 2 * rp
                    nc.vector.tensor_add(
                        outst[:, he * C : (he + 2) * C],
                        ps[:],
                        cursb[:, he * C : (he + 2) * C],
                    )
                # chunk output eviction to DRAM (bf16) via Pool SWDGE
                nc.gpsimd.dma_start(
                    out=out[:, ci * RCH : (ci + 1) * RCH, :].rearrange(
                        "w r c -> w (r c)"
                    ),
                    in_=outst[:, ci * RCH * C : (ci + 1) * RCH * C],
                )

    return nc


_NC = None
LAST_RESULT = None


def _get_nc():
    global _NC
    if _NC is None:
        _NC = _build_bass()
    return _NC


# ---------------------------------------------------------------------------
# Host-side shard prep
# ---------------------------------------------------------------------------


def _prep_core(attn_b, rv_b, cv_b, g0):
    """Build one core's in_map. attn_b/rv_b/cv_b: [H, W, ...] for one batch;
    g0: first output row of the shard."""
    # ref with 2-row halo, transposed to [w, r, c]
    refpad = np.zeros((HALO_R, W, C), np.float32)
    lo_g, hi_g = g0 - 2, g0 + HS + 2
    s0, s1 = max(lo_g, 0), min(hi_g, H)
    refpad[s0 - lo_g : s1 - lo_g] = rv_b[s0:s1]
    refhl = refpad.transpose(1, 0, 2).astype(bfloat16)  # [w, r, c]

    # current term pre-scaled by its attention weight (exact fp32 on host)
    cur = cv_b[g0 : g0 + HS] * attn_b[g0 : g0 + HS, :, 25:26]
    curhl = cur.transpose(1, 0, 2).astype(bfloat16)  # [w, h, c]

    # window-packed band values, 2-row sub-interleave: per (chunk, group,
    # partition, subregion) one contiguous 36-col x 5-band x 2-row window;
    # value for (w', j) at window col (w' mod 32) + j, zeros elsewhere
    A = attn_b[g0 : g0 + HS]  # [HS, W, 26]
    wp = np.arange(W)[:, None]
    jj = np.arange(5)[None, :]
    w = wp + jj - 2  # [w', j] out col
    valid = (w >= 0) & (w < W)
    wc = np.clip(w, 0, W - 1)
    kk = 5 * np.arange(5)[:, None] + 4 - np.arange(5)[None, :]  # [b, j]
    Aw = A[:, wc, :]  # [HS, w', j, 26]
    V = np.take_along_axis(Aw, kk.T[None, None, :, :], axis=3)  # [HS,w',j,b]
    V = V * valid[None, :, :, None]
    V7 = V.reshape(NCH, NSUB, 2, 4, 32, 5, 5)  # [c, s, r2, g, pl, j, b]
    W7 = np.zeros((NCH, 4, 32, NSUB, 36, 5, 2), np.float32)
    pl = np.arange(32)
    for j in range(5):
        rhs = V7[:, :, :, :, pl, j, :]  # [c, s, r2, g, pl, b]
        W7[:, :, pl, :, pl + j, :, :] = rhs.transpose(4, 0, 3, 1, 5, 2)
    return {
        "refhl": refhl,
        "curhl": curhl,
        "qb": W7.reshape(-1).astype(bfloat16),
    }


def kernel(attn, ref_value, current_ref_value):
    attn = np.asarray(attn, dtype=np.float32)
    rv = np.asarray(ref_value, dtype=np.float32)
    cv = np.asarray(current_ref_value, dtype=np.float32)

    nc = _get_nc()
    in_maps = []
    for core in range(NCORES):
        bb, half = divmod(core, 2)
        in_maps.append(_prep_core(attn[bb], rv[bb], cv[bb], half * HS))

    res = bass_utils.run_bass_kernel_spmd(nc, in_maps, core_ids=list(range(NCORES)))
    global LAST_RESULT
    LAST_RESULT = res

    out = np.empty((B, H, W, C), np.float32)
    for core in range(NCORES):
        bb, half = divmod(core, 2)
        dev = np.asarray(res.results[core]["out"]).astype(np.float32)  # [w, hs, c]
        out[bb, half * HS : (half + 1) * HS] = dev.transpose(1, 0, 2)
    return out


# revision 17
# speedup vs baseline: 1.0162x; 1.0162x over previous
"""Trainium2 Bass kernel for nn_AggregationLayer2 (5x5 spatially-varying
neighborhood aggregation, 26 slots: 25 spatial shifts + current value).

    out[b,h,w,c] = sum_k attn[b,h,w,k] * neighbor_k(ref_value)[c]
                 + attn[b,h,w,25] * current_ref_value[b,h,w,c]

Strategy (8 NeuronCores, SPMD), v3:
  - Shard: (batch, H-half) -> 8 shards of 64 output rows each; host ships
    ref rows with a 2-row zero halo, bf16.
  - Compute: per output row h and vertical offset b, the dj-contraction is
    a banded matmul out_row[w,c] += sum_{w'} BandT[w',w] * ref[h+b,w',c];
    5 PSUM-accumulated matmuls per output row on the TensorEngine.
  - Band storage: groups of R=8 rows interleaved so that element
    (col c, band b, row rr) sits at c*5R + b*R + rr. The PE reads a clean
    [128 x 128] AP at stride 5R; each partition w' holds its 25*R values
    in ONE contiguous 400B run at offset (w'+14)*5R -> the attn stream is
    shipped compact (0.4MB vs 2.9MB zero-inflated) with 128 descriptors
    per group instead of 25x that.
  - Zeros: band ring of 3 group-slots; gaps are memset once at startup
    (split across DVE/Act/Pool) and never dirtied - each reuse rewrites
    exactly the same value cells.
  - Output in bf16 (halves the output DMA); host upcasts.
  - Current term attn[...,25]*current is pre-scaled on the host (fp32) and
    folded in during 2-row PSUM evictions alternating DVE/Act.
  - PE warmup matmuls on scratch data raise the PE p-state during the
    input-DMA window so real matmuls run at full clock.
"""

import numpy as np
import ml_dtypes

import concourse.bass as bass
import concourse.mybir as mybir
from concourse.tile import TileContext
from concourse.tile_rust import add_dep_helper
from concourse.vector_clock import ScopedClock
from concourse import bass_utils

# ---------------------------------------------------------------------------
# Toolchain compat: this walrus build codegens at most one sync-wait command
# per instruction and rejects eq-mode waits on Drain ops. Replace the Tile
# tail barrier and split multi-waits onto standalone EventSemaphore waits.
# ---------------------------------------------------------------------------

_wsplit_counter = [0]


def _split_multi_waits(nc):
    for f in nc.m.functions:
        for bb in f.blocks:
            out = []
            changed = False
            for inst in bb.instructions:
                si = inst.sync_info
                if si is not None and len(si.on_wait) > 1:
                    waits = list(si.on_wait)
                    for w in waits[:-1]:
                        _wsplit_counter[0] += 1
                        ev = mybir.InstEventSemaphore(
                            name=f"WSPLIT-{_wsplit_counter[0]}",
                            engine=inst.engine,
                            ins=[],
                            outs=[],
                            sync_info=mybir.SyncInfo(on_wait=[w], on_update=[]),
                        )
                        out.append(ev)
                    si.on_wait = [waits[-1]]
                    changed = True
                out.append(inst)
            if changed:
                bb.instructions = out


def _drain_and_barrier_compat(self, tick_clock, wait_clock):
    nc = self.nc
    carrier = nc.sync.nop()
    wait_clock.add_sem_waits(
        carrier.ins, ScopedClock({None: tick_clock.global_clock})
    )
    waits = list(carrier.ins.sync_info.on_wait)
    if len(waits) > 1:
        carrier.ins.sync_info.on_wait = [waits[0]]
        engines = list(nc.engines.values())
        for idx, w in enumerate(waits[1:]):
            n = engines[idx % len(engines)].nop()
            n.ins.sync_info = mybir.SyncInfo(on_wait=[w], on_update=[])

    barrier_sem = nc.alloc_semaphore("tile_final_barrier")
    n_eng = len(nc.engines)
    for eng in nc.engines.values():
        eng.drain(fusable=False)
        eng.sem_inc(barrier_sem, 1)
        eng.wait_ge(barrier_sem, n_eng)
    for _ in range(4):
        nc.gpsimd.nop()
    nc.gpsimd.sem_clear(barrier_sem)

    popped = nc._tile_sem_poison_stack.pop()
    assert popped is self._sem_poison
    nc.clear_and_free_semaphores(list(self.sems.allocated().values()))


_orig_tc_exit = TileContext.__exit__


def _patched_tc_exit(self, exc_type, exc_value, traceback):
    r = _orig_tc_exit(self, exc_type, exc_value, traceback)
    if not exc_type:
        _split_multi_waits(self.nc)
    return r


def _install_tilefix():
    TileContext._drain_and_barrier = _drain_and_barrier_compat
    TileContext.__exit__ = _patched_tc_exit


_install_tilefix()


def _install_ntff_hook():
    """The image's antenv lacks axon_hooks; provide it and register the
    ctypes NTFF profiling hook so BASS_TRACE=1 yields HW exec times."""
    import sys
    import types

    if "antenv.axon_hooks" not in sys.modules:
        mod = types.ModuleType("antenv.axon_hooks")
        holder = [None]
        mod.set_axon_ntff_profile_hook = lambda h: holder.__setitem__(0, h)
        mod.get_axon_ntff_profile_hook = lambda: holder[0]
        sys.modules["antenv.axon_hooks"] = mod
        try:
            import antenv

            antenv.axon_hooks = mod
        except ImportError:
            pass
    try:
        from antenv.axon_hooks import (
            get_axon_ntff_profile_hook,
            set_axon_ntff_profile_hook,
        )

        if get_axon_ntff_profile_hook() is None:
            from trn_agent_boot.trn_boot import _ntff_profile_via_ctypes

            set_axon_ntff_profile_hook(
                _ntff_profile_via_ctypes("/opt/axon/libaxon_pjrt.so")
            )
    except Exception:
        pass

    # artifact upload needs external storage; degrade to local-only
    def _no_upload(tmpdir):
        return tmpdir

    bass_utils.upload_artifacts = _no_upload


_install_ntff_hook()

# ---------------------------------------------------------------------------
# Problem geometry (hardcoded per the harness contract)
# ---------------------------------------------------------------------------

B, H, W, C = 4, 128, 128, 64
NCORES = 8
HS = H // 2          # 64 output rows per shard
HALO_R = HS + 4      # 68 ref rows incl 2-row halo
RCH = 16             # rows per chunk
NCH = HS // RCH      # 4 chunks
RING = 4             # band slots (one per chunk, no reuse)
NSUB = 8             # 2-row subregions per chunk
SRW = 10             # col stride elems (5 bands x 2 rows)
SUBR = 160 * SRW     # 1600: per-partition elems per subregion
FGC = NSUB * SUBR    # 12800: per-partition elems per chunk region
WINS = 36 * SRW      # 360: window elems per (partition, subregion)
NBAND = 5
WARMN = 8            # PE warmup matmuls (N=512 each)

BF16 = mybir.dt.bfloat16
F32 = mybir.dt.float32
U32 = mybir.dt.uint32

bfloat16 = ml_dtypes.bfloat16

# ref piece row ranges (halo rows); chunk c needs rows <= c*RCH + RCH + 4
REF_PIECES = [(0, 22), (22, 38), (38, 54), (54, 68)]


def _build_bass():
    nc = bass.Bass()
    refhl = nc.dram_tensor("refhl", [W, HALO_R, C], BF16, kind="ExternalInput")
    curhl = nc.dram_tensor("curhl", [W, HS, C], BF16, kind="ExternalInput")
    qb = nc.dram_tensor("qb", [NCH * W * NSUB * WINS], BF16, kind="ExternalInput")
    out = nc.dram_tensor("out", [W, HS, C], BF16, kind="ExternalOutput")

    with TileContext(nc) as tc:
        with (
            tc.tile_pool(name="sb", bufs=1) as pool,
            tc.tile_pool(name="band", bufs=1) as band_pool,
            tc.tile_pool(name="ps", bufs=7, space="PSUM") as psum_pool,
            tc.tile_pool(name="wp", bufs=1, space="PSUM") as warm_pool,
        ):
            refsb = pool.tile([W, HALO_R * C], BF16, tag="refsb")
            cursb = pool.tile([W, HS * C], BF16, tag="cursb")
            outst = pool.tile([W, HS * C], BF16, tag="outst")
            scratch = pool.tile([W, 512], BF16, tag="scratch")

            # --- PE warmup: raise p-state while inputs stream in ---
            nc.vector.memset(scratch[:].bitcast(U32), 0)
            warm_ps = warm_pool.tile([W, 512], F32, tag="warm")
            for _ in range(WARMN):
                nc.tensor.matmul(
                    warm_ps[:],
                    scratch[:, 0:128],
                    scratch[:],
                    start=True,
                    stop=True,
                )

            refv = refhl[:].rearrange("w r c -> w (r c)")
            curv = curhl[:].rearrange("w r c -> w (r c)")

            # chunk-0 qb prefetch into a staging tile (no memset dep -> its
            # HBM fetch overlaps the band memsets; the windowed scatter
            # below is then a short SBUF->SBUF hop)
            stage0 = pool.tile([W, NSUB * WINS], BF16, tag="stage0")
            nc.sync.dma_start(
                out=stage0[:],
                in_=qb[0 : W * NSUB * WINS].rearrange("(p x) -> p x", p=W),
            )

            # --- band slots, one per chunk (no reuse -> no WAR chains).
            # Only the PE-read col range [16,144) of each subregion needs
            # zeros; window cells are rewritten by the scatters. Slot 0
            # memsets 3-way (DVE/Act/Pool) so chunk 0 unblocks fastest;
            # the rest split DVE/Pool halves, in need order.
            ring = []
            for s in range(RING):
                t = band_pool.tile([W, FGC], BF16, tag=f"band{s}")
                u = t[:].rearrange("p (s x) -> p s x", s=NSUB).bitcast(U32)
                if s == 0:
                    nc.vector.memset(u[:, 0:3, 80:720], 0)
                    nc.scalar.memzero(u[:, 3:5, 80:720])
                    nc.gpsimd.memset(u[:, 5:, 80:720], 0)
                else:
                    nc.vector.memset(u[:, 0:4, 80:720], 0)
                    nc.gpsimd.memset(u[:, 4:, 80:720], 0)
                ring.append(t)

            # ref/cur issue on Act after its slot-0 memzero
            nc.scalar.dma_start(
                out=refsb[:, 0 : 22 * C], in_=refv[:, 0 : 22 * C]
            )
            nc.scalar.dma_start(
                out=cursb[:, 0 : 32 * C], in_=curv[:, 0 : 32 * C]
            )
            a, b = REF_PIECES[1]
            nc.scalar.dma_start(
                out=refsb[:, a * C : b * C], in_=refv[:, a * C : b * C]
            )

            for ci in range(NCH):
                slot = ring[ci]
                for g in range(4):
                    c0 = (32 * g + 14) * SRW
                    dst = slot[32 * g : 32 * g + 32, :].rearrange(
                        "p (s x) -> p s x", s=NSUB
                    )[:, :, c0 : c0 + WINS]
                    if ci == 0:
                        # short SBUF->SBUF hop from the prefetched stage
                        src = stage0[32 * g : 32 * g + 32, :].rearrange(
                            "p (s j) -> p s j", j=WINS
                        )
                        nc.sync.dma_start(out=dst, in_=src)
                    else:
                        base = (ci * 4 + g) * 32 * NSUB * WINS
                        src = qb[base : base + 32 * NSUB * WINS].rearrange(
                            "(p s j) -> p s j", p=32, j=WINS
                        )
                        nc.sync.dma_start(out=dst, in_=src)
                if ci == 0:
                    # later ref/cur pieces, behind chunk-0/1 traffic
                    for i in (2, 3):
                        a, b = REF_PIECES[i]
                        nc.gpsimd.dma_start(
                            out=refsb[:, a * C : b * C],
                            in_=refv[:, a * C : b * C],
                        )
                    nc.gpsimd.dma_start(
                        out=cursb[:, 32 * C :], in_=curv[:, 32 * C :]
                    )

                view = slot[:].rearrange(
                    "p (s c b r) -> p s c b r", s=NSUB, b=NBAND, r=2
                )
                for rp in range(RCH // 2):
                    ps = psum_pool.tile([W, 2 * C], F32, tag="ps")
                    for sub in range(2):
                        rr = 2 * rp + sub
                        h = ci * RCH + rr
                        for b in range(NBAND):
                            lhsT = view[:, rr // 2, 16:144, b, rr % 2]
                            rhs = refsb[:, (h + b) * C : (h + b + 1) * C]
                            nc.tensor.matmul(
                                ps[:, sub * C : (sub + 1) * C],
                                lhsT,
                                rhs,
                                start=(b == 0),
                                stop=(b == NBAND - 1),
                            )
                    he = ci * RCH + 2 * rp
                    nc.vector.tensor_add(
                        outst[:, he * C : (he + 2) * C],
                        ps[:],
                        cursb[:, he * C : (he + 2) * C],
                    )
                # chunk output eviction to DRAM (bf16); last chunk on the
                # idle SP HWDGE queue for a shorter tail
                oeng = nc.sync if ci == NCH - 1 else nc.gpsimd
                oeng.dma_start(
                    out=out[:, ci * RCH : (ci + 1) * RCH, :].rearrange(
                        "w r c -> w (r c)"
                    ),
                    in_=outst[:, ci * RCH * C : (ci + 1) * RCH * C],
                )

    return nc


_NC = None
LAST_RESULT = None


def _get_nc():
    global _NC
    if _NC is None:
        _NC = _build_bass()
    return _NC


# ---------------------------------------------------------------------------
# Host-side shard prep
# ---------------------------------------------------------------------------


def _prep_core(attn_b, rv_b, cv_b, g0):
    """Build one core's in_map. attn_b/rv_b/cv_b: [H, W, ...] for one batch;
    g0: first output row of the shard."""
    # ref with 2-row halo, transposed to [w, r, c]
    refpad = np.zeros((HALO_R, W, C), np.float32)
    lo_g, hi_g = g0 - 2, g0 + HS + 2
    s0, s1 = max(lo_g, 0), min(hi_g, H)
    refpad[s0 - lo_g : s1 - lo_g] = rv_b[s0:s1]
    refhl = refpad.transpose(1, 0, 2).astype(bfloat16)  # [w, r, c]

    # current term pre-scaled by its attention weight (exact fp32 on host)
    cur = cv_b[g0 : g0 + HS] * attn_b[g0 : g0 + HS, :, 25:26]
    curhl = cur.transpose(1, 0, 2).astype(bfloat16)  # [w, h, c]

    # window-packed band values, 2-row sub-interleave: per (chunk, group,
    # partition, subregion) one contiguous 36-col x 5-band x 2-row window;
    # value for (w', j) at window col (w' mod 32) + j, zeros elsewhere
    A = attn_b[g0 : g0 + HS]  # [HS, W, 26]
    wp = np.arange(W)[:, None]
    jj = np.arange(5)[None, :]
    w = wp + jj - 2  # [w', j] out col
    valid = (w >= 0) & (w < W)
    wc = np.clip(w, 0, W - 1)
    kk = 5 * np.arange(5)[:, None] + 4 - np.arange(5)[None, :]  # [b, j]
    Aw = A[:, wc, :]  # [HS, w', j, 26]
    V = np.take_along_axis(Aw, kk.T[None, None, :, :], axis=3)  # [HS,w',j,b]
    V = V * valid[None, :, :, None]
    V7 = V.reshape(NCH, NSUB, 2, 4, 32, 5, 5)  # [c, s, r2, g, pl, j, b]
    W7 = np.zeros((NCH, 4, 32, NSUB, 36, 5, 2), np.float32)
    pl = np.arange(32)
    for j in range(5):
        rhs = V7[:, :, :, :, pl, j, :]  # [c, s, r2, g, pl, b]
        W7[:, :, pl, :, pl + j, :, :] = rhs.transpose(4, 0, 3, 1, 5, 2)
    return {
        "refhl": refhl,
        "curhl": curhl,
        "qb": W7.reshape(-1).astype(bfloat16),
    }


def kernel(attn, ref_value, current_ref_value):
    attn = np.asarray(attn, dtype=np.float32)
    rv = np.asarray(ref_value, dtype=np.float32)
    cv = np.asarray(current_ref_value, dtype=np.float32)

    nc = _get_nc()
    in_maps = []
    for core in range(NCORES):
        bb, half = divmod(core, 2)
        in_maps.append(_prep_core(attn[bb], rv[bb], cv[bb], half * HS))

    res = bass_utils.run_bass_kernel_spmd(nc, in_maps, core_ids=list(range(NCORES)))
    global LAST_RESULT
    LAST_RESULT = res

    out = np.empty((B, H, W, C), np.float32)
    for core in range(NCORES):
        bb, half = divmod(core, 2)
        dev = np.asarray(res.results[core]["out"]).astype(np.float32)  # [w, hs, c]
        out[bb, half * HS : (half + 1) * HS] = dev.transpose(1, 0, 2)
    return out


# revision 19
# speedup vs baseline: 1.0838x; 1.0665x over previous
"""Trainium2 Bass kernel for nn_AggregationLayer2 (5x5 spatially-varying
neighborhood aggregation, 26 slots: 25 spatial shifts + current value).

    out[b,h,w,c] = sum_k attn[b,h,w,k] * neighbor_k(ref_value)[c]
                 + attn[b,h,w,25] * current_ref_value[b,h,w,c]

Strategy (8 NeuronCores, SPMD), v3:
  - Shard: (batch, H-half) -> 8 shards of 64 output rows each; host ships
    ref rows with a 2-row zero halo, bf16.
  - Compute: per output row h and vertical offset b, the dj-contraction is
    a banded matmul out_row[w,c] += sum_{w'} BandT[w',w] * ref[h+b,w',c];
    5 PSUM-accumulated matmuls per output row on the TensorEngine.
  - Band storage: groups of R=8 rows interleaved so that element
    (col c, band b, row rr) sits at c*5R + b*R + rr. The PE reads a clean
    [128 x 128] AP at stride 5R; each partition w' holds its 25*R values
    in ONE contiguous 400B run at offset (w'+14)*5R -> the attn stream is
    shipped compact (0.4MB vs 2.9MB zero-inflated) with 128 descriptors
    per group instead of 25x that.
  - Zeros: band ring of 3 group-slots; gaps are memset once at startup
    (split across DVE/Act/Pool) and never dirtied - each reuse rewrites
    exactly the same value cells.
  - Output in bf16 (halves the output DMA); host upcasts.
  - Current term attn[...,25]*current is pre-scaled on the host (fp32) and
    folded in during 2-row PSUM evictions alternating DVE/Act.
  - PE warmup matmuls on scratch data raise the PE p-state during the
    input-DMA window so real matmuls run at full clock.
"""

import numpy as np
import ml_dtypes

import concourse.bass as bass
import concourse.mybir as mybir
from concourse.tile import TileContext
from concourse.tile_rust import add_dep_helper
from concourse.vector_clock import ScopedClock
from concourse import bass_utils

# ---------------------------------------------------------------------------
# Toolchain compat: this walrus build codegens at most one sync-wait command
# per instruction and rejects eq-mode waits on Drain ops. Replace the Tile
# tail barrier and split multi-waits onto standalone EventSemaphore waits.
# ---------------------------------------------------------------------------

_wsplit_counter = [0]


def _split_multi_waits(nc):
    for f in nc.m.functions:
        for bb in f.blocks:
            out = []
            changed = False
            for inst in bb.instructions:
                si = inst.sync_info
                if si is not None and len(si.on_wait) > 1:
                    waits = list(si.on_wait)
                    for w in waits[:-1]:
                        _wsplit_counter[0] += 1
                        ev = mybir.InstEventSemaphore(
                            name=f"WSPLIT-{_wsplit_counter[0]}",
                            engine=inst.engine,
                            ins=[],
                            outs=[],
                            sync_info=mybir.SyncInfo(on_wait=[w], on_update=[]),
                        )
                        out.append(ev)
                    si.on_wait = [waits[-1]]
                    changed = True
                out.append(inst)
            if changed:
                bb.instructions = out


def _drain_and_barrier_compat(self, tick_clock, wait_clock):
    nc = self.nc
    carrier = nc.sync.nop()
    wait_clock.add_sem_waits(
        carrier.ins, ScopedClock({None: tick_clock.global_clock})
    )
    waits = list(carrier.ins.sync_info.on_wait)
    if len(waits) > 1:
        carrier.ins.sync_info.on_wait = [waits[0]]
        engines = list(nc.engines.values())
        for idx, w in enumerate(waits[1:]):
            n = engines[idx % len(engines)].nop()
            n.ins.sync_info = mybir.SyncInfo(on_wait=[w], on_update=[])

    barrier_sem = nc.alloc_semaphore("tile_final_barrier")
    n_eng = len(nc.engines)
    for eng in nc.engines.values():
        eng.drain(fusable=False)
        eng.sem_inc(barrier_sem, 1)
        eng.wait_ge(barrier_sem, n_eng)
    for _ in range(4):
        nc.gpsimd.nop()
    nc.gpsimd.sem_clear(barrier_sem)

    popped = nc._tile_sem_poison_stack.pop()
    assert popped is self._sem_poison
    nc.clear_and_free_semaphores(list(self.sems.allocated().values()))


_orig_tc_exit = TileContext.__exit__


def _patched_tc_exit(self, exc_type, exc_value, traceback):
    r = _orig_tc_exit(self, exc_type, exc_value, traceback)
    if not exc_type:
        _split_multi_waits(self.nc)
    return r


def _install_tilefix():
    TileContext._drain_and_barrier = _drain_and_barrier_compat
    TileContext.__exit__ = _patched_tc_exit


_install_tilefix()


def _install_ntff_hook():
    """The image's antenv lacks axon_hooks; provide it and register the
    ctypes NTFF profiling hook so BASS_TRACE=1 yields HW exec times."""
    import sys
    import types

    if "antenv.axon_hooks" not in sys.modules:
        mod = types.ModuleType("antenv.axon_hooks")
        holder = [None]
        mod.set_axon_ntff_profile_hook = lambda h: holder.__setitem__(0, h)
        mod.get_axon_ntff_profile_hook = lambda: holder[0]
        sys.modules["antenv.axon_hooks"] = mod
        try:
            import antenv

            antenv.axon_hooks = mod
        except ImportError:
            pass
    try:
        from antenv.axon_hooks import (
            get_axon_ntff_profile_hook,
            set_axon_ntff_profile_hook,
        )

        if get_axon_ntff_profile_hook() is None:
            from trn_agent_boot.trn_boot import _ntff_profile_via_ctypes

            set_axon_ntff_profile_hook(
                _ntff_profile_via_ctypes("/opt/axon/libaxon_pjrt.so")
            )
    except Exception:
        pass

    # artifact upload needs external storage; degrade to local-only
    def _no_upload(tmpdir):
        return tmpdir

    bass_utils.upload_artifacts = _no_upload


_install_ntff_hook()

# ---------------------------------------------------------------------------
# Problem geometry (hardcoded per the harness contract)
# ---------------------------------------------------------------------------

B, H, W, C = 4, 128, 128, 64
NCORES = 8
HS = H // 2          # 64 output rows per shard
HALO_R = HS + 4      # 68 ref rows incl 2-row halo
RCH = 16             # rows per chunk
NCH = HS // RCH      # 4 chunks
RING = 3             # band slots (chunk 3 reuses slot 0)
NSUB = 8             # 2-row subregions per chunk
SRW = 10             # col stride elems (5 bands x 2 rows)
SUBR = 160 * SRW     # 1600: per-partition elems per subregion
FGC = NSUB * SUBR    # 12800: per-partition elems per chunk region
WINS = 36 * SRW      # 360: window elems per (partition, subregion)
NBAND = 5
WARMN = 8            # PE warmup matmuls (N=512 each)

BF16 = mybir.dt.bfloat16
F32 = mybir.dt.float32
U32 = mybir.dt.uint32

bfloat16 = ml_dtypes.bfloat16

# ref piece row ranges (halo rows); chunk c needs rows <= c*RCH + RCH + 4
REF_PIECES = [(0, 22), (22, 38), (38, 54), (54, 68)]


def _build_bass():
    nc = bass.Bass()
    refhl = nc.dram_tensor("refhl", [W, HALO_R, C], BF16, kind="ExternalInput")
    curhl = nc.dram_tensor("curhl", [W, HS, C], BF16, kind="ExternalInput")
    qb = nc.dram_tensor("qb", [NCH * W * NSUB * WINS], BF16, kind="ExternalInput")
    out = nc.dram_tensor("out", [W, HS, C], BF16, kind="ExternalOutput")

    with TileContext(nc) as tc:
        with (
            tc.tile_pool(name="sb", bufs=1) as pool,
            tc.tile_pool(name="band", bufs=1) as band_pool,
            tc.tile_pool(name="ps", bufs=7, space="PSUM") as psum_pool,
            tc.tile_pool(name="wp", bufs=1, space="PSUM") as warm_pool,
        ):
            refsb = pool.tile([W, HALO_R * C], BF16, tag="refsb")
            cursb = pool.tile([W, HS * C], BF16, tag="cursb")
            outst = pool.tile([W, HS * C], BF16, tag="outst")
            scratch = pool.tile([W, 512], BF16, tag="scratch")

            # --- PE warmup: raise p-state while inputs stream in ---
            nc.vector.memset(scratch[:].bitcast(U32), 0)
            warm_ps = warm_pool.tile([W, 512], F32, tag="warm")
            for _ in range(WARMN):
                nc.tensor.matmul(
                    warm_ps[:],
                    scratch[:, 0:128],
                    scratch[:],
                    start=True,
                    stop=True,
                )

            refv = refhl[:].rearrange("w r c -> w (r c)")
            curv = curhl[:].rearrange("w r c -> w (r c)")


            # --- band slots: 3 slots, chunk 3 reuses slot 0 via pool
            # rotation (write-after-read handled by the pool). Zeros are
            # memset once per slot over the PE-read col range [16,144);
            # window cells are rewritten by the scatters. Hand-scheduled
            # across DVE/Act/Pool so slot k is zeroed just before chunk k's
            # scatter fires, and chunk 0 unblocks fastest.
            def slot_tile(s):
                return band_pool.tile([W, FGC], BF16, name=f"band{s}", tag=f"band{s}")

            def win_dst(slot, g):
                c0 = (32 * g + 14) * SRW
                return slot[32 * g : 32 * g + 32, :].rearrange(
                    "p (s x) -> p s x", s=NSUB
                )[:, :, c0 : c0 + WINS]

            def win_src(ci, g):
                base = (ci * 4 + g) * 32 * NSUB * WINS
                return qb[base : base + 32 * NSUB * WINS].rearrange(
                    "(p s j) -> p s j", p=32, j=WINS
                )

            s0 = slot_tile(0)
            u0 = s0[:].rearrange("p (s x) -> p s x", s=NSUB).bitcast(U32)
            nc.vector.memset(u0[:, 0:2, 80:720], 0)
            nc.scalar.memzero(u0[:, 2:5, 80:720])
            nc.gpsimd.memset(u0[:, 5:, 80:720], 0)

            # chunk-0 inputs: scatters fan out over SP/SP/Act/Pool so their
            # DGE waits resolve in parallel the moment slot 0 is zeroed
            nc.scalar.dma_start(
                out=refsb[:, 0 : 22 * C], in_=refv[:, 0 : 22 * C]
            )
            nc.sync.dma_start(out=win_dst(s0, 0), in_=win_src(0, 0))
            nc.sync.dma_start(out=win_dst(s0, 1), in_=win_src(0, 1))
            nc.scalar.dma_start(out=win_dst(s0, 2), in_=win_src(0, 2))
            nc.gpsimd.dma_start(out=win_dst(s0, 3), in_=win_src(0, 3))
            nc.scalar.dma_start(
                out=cursb[:, 0 : 32 * C], in_=curv[:, 0 : 32 * C]
            )
            a, b = REF_PIECES[1]
            nc.scalar.dma_start(
                out=refsb[:, a * C : b * C], in_=refv[:, a * C : b * C]
            )

            s1 = slot_tile(1)
            u1 = s1[:].rearrange("p (s x) -> p s x", s=NSUB).bitcast(U32)
            nc.vector.memset(u1[:, 0:4, 80:720], 0)
            nc.gpsimd.memset(u1[:, 4:, 80:720], 0)
            s2 = slot_tile(2)
            u2 = s2[:].rearrange("p (s x) -> p s x", s=NSUB).bitcast(U32)
            nc.vector.memset(u2[:, 0:3, 80:720], 0)
            nc.gpsimd.memset(u2[:, 3:, 80:720], 0)
            ring = [s0, s1, s2]

            for ci in range(NCH):
                if ci == 0:
                    slot = ring[0]
                elif ci < RING:
                    slot = ring[ci]
                else:
                    slot = slot_tile(0)  # chunk 3: pool-rotated reuse of slot 0
                if ci > 0:
                    for g in range(4):
                        nc.sync.dma_start(
                            out=win_dst(slot, g), in_=win_src(ci, g)
                        )
                if ci == 1:
                    # later ref/cur pieces, behind chunk-0/1 traffic
                    for i in (2, 3):
                        a, b = REF_PIECES[i]
                        nc.gpsimd.dma_start(
                            out=refsb[:, a * C : b * C],
                            in_=refv[:, a * C : b * C],
                        )
                    nc.gpsimd.dma_start(
                        out=cursb[:, 32 * C :], in_=curv[:, 32 * C :]
                    )

                view = slot[:].rearrange(
                    "p (s c b r) -> p s c b r", s=NSUB, b=NBAND, r=2
                )
                for rp in range(RCH // 2):
                    ps = psum_pool.tile([W, 2 * C], F32, tag="ps")
                    for sub in range(2):
                        rr = 2 * rp + sub
                        h = ci * RCH + rr
                        for b in range(NBAND):
                            lhsT = view[:, rr // 2, 16:144, b, rr % 2]
                            rhs = refsb[:, (h + b) * C : (h + b + 1) * C]
                            nc.tensor.matmul(
                                ps[:, sub * C : (sub + 1) * C],
                                lhsT,
                                rhs,
                                start=(b == 0),
                                stop=(b == NBAND - 1),
                            )
                    he = ci * RCH + 2 * rp
                    nc.vector.tensor_add(
                        outst[:, he * C : (he + 2) * C],
                        ps[:],
                        cursb[:, he * C : (he + 2) * C],
                    )
                # chunk output eviction to DRAM (bf16); last chunk on the
                # idle SP HWDGE queue for a shorter tail
                oeng = nc.sync if ci == NCH - 1 else nc.gpsimd
                oeng.dma_start(
                    out=out[:, ci * RCH : (ci + 1) * RCH, :].rearrange(
                        "w r c -> w (r c)"
                    ),
                    in_=outst[:, ci * RCH * C : (ci + 1) * RCH * C],
                )

    return nc


_NC = None
LAST_RESULT = None


def _get_nc():
    global _NC
    if _NC is None:
        _NC = _build_bass()
    return _NC


# ---------------------------------------------------------------------------
# Host-side shard prep
# ---------------------------------------------------------------------------


def _prep_core(attn_b, rv_b, cv_b, g0):
    """Build one core's in_map. attn_b/rv_b/cv_b: [H, W, ...] for one batch;
    g0: first output row of the shard."""
    # ref with 2-row halo, transposed to [w, r, c]
    refpad = np.zeros((HALO_R, W, C), np.float32)
    lo_g, hi_g = g0 - 2, g0 + HS + 2
    s0, s1 = max(lo_g, 0), min(hi_g, H)
    refpad[s0 - lo_g : s1 - lo_g] = rv_b[s0:s1]
    refhl = refpad.transpose(1, 0, 2).astype(bfloat16)  # [w, r, c]

    # current term pre-scaled by its attention weight (exact fp32 on host)
    cur = cv_b[g0 : g0 + HS] * attn_b[g0 : g0 + HS, :, 25:26]
    curhl = cur.transpose(1, 0, 2).astype(bfloat16)  # [w, h, c]

    # window-packed band values, 2-row sub-interleave: per (chunk, group,
    # partition, subregion) one contiguous 36-col x 5-band x 2-row window;
    # value for (w', j) at window col (w' mod 32) + j, zeros elsewhere
    A = attn_b[g0 : g0 + HS]  # [HS, W, 26]
    wp = np.arange(W)[:, None]
    jj = np.arange(5)[None, :]
    w = wp + jj - 2  # [w', j] out col
    valid = (w >= 0) & (w < W)
    wc = np.clip(w, 0, W - 1)
    kk = 5 * np.arange(5)[:, None] + 4 - np.arange(5)[None, :]  # [b, j]
    Aw = A[:, wc, :]  # [HS, w', j, 26]
    V = np.take_along_axis(Aw, kk.T[None, None, :, :], axis=3)  # [HS,w',j,b]
    V = V * valid[None, :, :, None]
    V7 = V.reshape(NCH, NSUB, 2, 4, 32, 5, 5)  # [c, s, r2, g, pl, j, b]
    W7 = np.zeros((NCH, 4, 32, NSUB, 36, 5, 2), np.float32)
    pl = np.arange(32)
    for j in range(5):
        rhs = V7[:, :, :, :, pl, j, :]  # [c, s, r2, g, pl, b]
        W7[:, :, pl, :, pl + j, :, :] = rhs.transpose(4, 0, 3, 1, 5, 2)
    return {
        "refhl": refhl,
        "curhl": curhl,
        "qb": W7.reshape(-1).astype(bfloat16),
    }


def kernel(attn, ref_value, current_ref_value):
    attn = np.asarray(attn, dtype=np.float32)
    rv = np.asarray(ref_value, dtype=np.float32)
    cv = np.asarray(current_ref_value, dtype=np.float32)

    nc = _get_nc()
    in_maps = []
    for core in range(NCORES):
        bb, half = divmod(core, 2)
        in_maps.append(_prep_core(attn[bb], rv[bb], cv[bb], half * HS))

    res = bass_utils.run_bass_kernel_spmd(nc, in_maps, core_ids=list(range(NCORES)))
    global LAST_RESULT
    LAST_RESULT = res

    out = np.empty((B, H, W, C), np.float32)
    for core in range(NCORES):
        bb, half = divmod(core, 2)
        dev = np.asarray(res.results[core]["out"]).astype(np.float32)  # [w, hs, c]
        out[bb, half * HS : (half + 1) * HS] = dev.transpose(1, 0, 2)
    return out


# revision 20
# speedup vs baseline: 1.1731x; 1.0824x over previous
"""Trainium2 Bass kernel for nn_AggregationLayer2 (5x5 spatially-varying
neighborhood aggregation, 26 slots: 25 spatial shifts + current value).

    out[b,h,w,c] = sum_k attn[b,h,w,k] * neighbor_k(ref_value)[c]
                 + attn[b,h,w,25] * current_ref_value[b,h,w,c]

Strategy (8 NeuronCores, SPMD), v3:
  - Shard: (batch, H-half) -> 8 shards of 64 output rows each; host ships
    ref rows with a 2-row zero halo, bf16.
  - Compute: per output row h and vertical offset b, the dj-contraction is
    a banded matmul out_row[w,c] += sum_{w'} BandT[w',w] * ref[h+b,w',c];
    5 PSUM-accumulated matmuls per output row on the TensorEngine.
  - Band storage: groups of R=8 rows interleaved so that element
    (col c, band b, row rr) sits at c*5R + b*R + rr. The PE reads a clean
    [128 x 128] AP at stride 5R; each partition w' holds its 25*R values
    in ONE contiguous 400B run at offset (w'+14)*5R -> the attn stream is
    shipped compact (0.4MB vs 2.9MB zero-inflated) with 128 descriptors
    per group instead of 25x that.
  - Zeros: band ring of 3 group-slots; gaps are memset once at startup
    (split across DVE/Act/Pool) and never dirtied - each reuse rewrites
    exactly the same value cells.
  - Output in bf16 (halves the output DMA); host upcasts.
  - Current term attn[...,25]*current is pre-scaled on the host (fp32) and
    folded in during 2-row PSUM evictions alternating DVE/Act.
  - PE warmup matmuls on scratch data raise the PE p-state during the
    input-DMA window so real matmuls run at full clock.
"""

import numpy as np
import ml_dtypes

import concourse.bass as bass
import concourse.mybir as mybir
from concourse.tile import TileContext
from concourse.tile_rust import add_dep_helper
from concourse.vector_clock import ScopedClock
from concourse import bass_utils

# ---------------------------------------------------------------------------
# Toolchain compat: this walrus build codegens at most one sync-wait command
# per instruction and rejects eq-mode waits on Drain ops. Replace the Tile
# tail barrier and split multi-waits onto standalone EventSemaphore waits.
# ---------------------------------------------------------------------------

_wsplit_counter = [0]


def _split_multi_waits(nc):
    for f in nc.m.functions:
        for bb in f.blocks:
            out = []
            changed = False
            for inst in bb.instructions:
                si = inst.sync_info
                if si is not None and len(si.on_wait) > 1:
                    waits = list(si.on_wait)
                    for w in waits[:-1]:
                        _wsplit_counter[0] += 1
                        ev = mybir.InstEventSemaphore(
                            name=f"WSPLIT-{_wsplit_counter[0]}",
                            engine=inst.engine,
                            ins=[],
                            outs=[],
                            sync_info=mybir.SyncInfo(on_wait=[w], on_update=[]),
                        )
                        out.append(ev)
                    si.on_wait = [waits[-1]]
                    changed = True
                out.append(inst)
            if changed:
                bb.instructions = out


def _drain_and_barrier_compat(self, tick_clock, wait_clock):
    nc = self.nc
    carrier = nc.sync.nop()
    wait_clock.add_sem_waits(
        carrier.ins, ScopedClock({None: tick_clock.global_clock})
    )
    waits = list(carrier.ins.sync_info.on_wait)
    if len(waits) > 1:
        carrier.ins.sync_info.on_wait = [waits[0]]
        engines = list(nc.engines.values())
        for idx, w in enumerate(waits[1:]):
            n = engines[idx % len(engines)].nop()
            n.ins.sync_info = mybir.SyncInfo(on_wait=[w], on_update=[])

    barrier_sem = nc.alloc_semaphore("tile_final_barrier")
    n_eng = len(nc.engines)
    for eng in nc.engines.values():
        eng.drain(fusable=False)
        eng.sem_inc(barrier_sem, 1)
        eng.wait_ge(barrier_sem, n_eng)
    for _ in range(4):
        nc.gpsimd.nop()
    nc.gpsimd.sem_clear(barrier_sem)

    popped = nc._tile_sem_poison_stack.pop()
    assert popped is self._sem_poison
    nc.clear_and_free_semaphores(list(self.sems.allocated().values()))


_orig_tc_exit = TileContext.__exit__


def _patched_tc_exit(self, exc_type, exc_value, traceback):
    r = _orig_tc_exit(self, exc_type, exc_value, traceback)
    if not exc_type:
        _split_multi_waits(self.nc)
    return r


def _install_tilefix():
    TileContext._drain_and_barrier = _drain_and_barrier_compat
    TileContext.__exit__ = _patched_tc_exit


_install_tilefix()


def _install_ntff_hook():
    """The image's antenv lacks axon_hooks; provide it and register the
    ctypes NTFF profiling hook so BASS_TRACE=1 yields HW exec times."""
    import sys
    import types

    if "antenv.axon_hooks" not in sys.modules:
        mod = types.ModuleType("antenv.axon_hooks")
        holder = [None]
        mod.set_axon_ntff_profile_hook = lambda h: holder.__setitem__(0, h)
        mod.get_axon_ntff_profile_hook = lambda: holder[0]
        sys.modules["antenv.axon_hooks"] = mod
        try:
            import antenv

            antenv.axon_hooks = mod
        except ImportError:
            pass
    try:
        from antenv.axon_hooks import (
            get_axon_ntff_profile_hook,
            set_axon_ntff_profile_hook,
        )

        if get_axon_ntff_profile_hook() is None:
            from trn_agent_boot.trn_boot import _ntff_profile_via_ctypes

            set_axon_ntff_profile_hook(
                _ntff_profile_via_ctypes("/opt/axon/libaxon_pjrt.so")
            )
    except Exception:
        pass

    # artifact upload needs external storage; degrade to local-only
    def _no_upload(tmpdir):
        return tmpdir

    bass_utils.upload_artifacts = _no_upload


_install_ntff_hook()

# ---------------------------------------------------------------------------
# Problem geometry (hardcoded per the harness contract)
# ---------------------------------------------------------------------------

B, H, W, C = 4, 128, 128, 64
NCORES = 8
HS = H // 2          # 64 output rows per shard
HALO_R = HS + 4      # 68 ref rows incl 2-row halo
RCH = 16             # rows per chunk
NCH = HS // RCH      # 4 chunks
RING = 3             # band slots (chunk 3 reuses slot 0)
NSUB = 8             # 2-row subregions per chunk
SRW = 10             # col stride elems (5 bands x 2 rows)
SUBR = 160 * SRW     # 1600: per-partition elems per subregion
FGC = NSUB * SUBR    # 12800: per-partition elems per chunk region
WINS = 36 * SRW      # 360: window elems per (partition, subregion)
NBAND = 5
WARMN = 5            # PE warmup matmuls (N=512 each)

BF16 = mybir.dt.bfloat16
F32 = mybir.dt.float32
U32 = mybir.dt.uint32

bfloat16 = ml_dtypes.bfloat16

# ref piece row ranges (halo rows); chunk c needs rows <= c*RCH + RCH + 4
REF_PIECES = [(0, 22), (22, 38), (38, 54), (54, 68)]


def _build_bass():
    nc = bass.Bass()
    refhl = nc.dram_tensor("refhl", [W, HALO_R, C], BF16, kind="ExternalInput")
    curhl = nc.dram_tensor("curhl", [W, HS, C], BF16, kind="ExternalInput")
    qb = nc.dram_tensor("qb", [NCH * W * NSUB * WINS], BF16, kind="ExternalInput")
    out = nc.dram_tensor("out", [W, HS, C], BF16, kind="ExternalOutput")

    with TileContext(nc) as tc:
        with (
            tc.tile_pool(name="sb", bufs=1) as pool,
            tc.tile_pool(name="band", bufs=1) as band_pool,
            tc.tile_pool(name="ps", bufs=7, space="PSUM") as psum_pool,
            tc.tile_pool(name="wp", bufs=1, space="PSUM") as warm_pool,
        ):
            refsb = pool.tile([W, HALO_R * C], BF16, tag="refsb")
            cursb = pool.tile([W, HS * C], BF16, tag="cursb")
            outst = pool.tile([W, HS * C], BF16, tag="outst")
            scratch = pool.tile([W, 512], BF16, tag="scratch")

            # --- PE warmup: raise p-state while inputs stream in ---
            nc.vector.memset(scratch[:].bitcast(U32), 0)
            warm_ps = warm_pool.tile([W, 512], F32, tag="warm")
            for _ in range(WARMN):
                nc.tensor.matmul(
                    warm_ps[:],
                    scratch[:, 0:128],
                    scratch[:],
                    start=True,
                    stop=True,
                )

            refv = refhl[:].rearrange("w r c -> w (r c)")
            curv = curhl[:].rearrange("w r c -> w (r c)")


            # --- band slots: 3 slots, chunk 3 reuses slot 0 via pool
            # rotation. Zeros memset once per slot over the PE-read col
            # range; slot 0 in two subregion phases so chunk 0's first
            # windows scatter while its second half is still zeroing.
            # DVE+Pool only (no Act memsets -> no act-table preamble load).
            def slot_tile(s):
                return band_pool.tile([W, FGC], BF16, name=f"band{s}", tag=f"band{s}")

            def win_dst(slot, g, sa, sb):
                c0 = (32 * g + 14) * SRW
                return slot[32 * g : 32 * g + 32, :].rearrange(
                    "p (s x) -> p s x", s=NSUB
                )[:, sa:sb, c0 : c0 + WINS]

            s0 = slot_tile(0)
            u0 = s0[:].rearrange("p (s x) -> p s x", s=NSUB).bitcast(U32)
            nc.vector.memset(u0[:, 0:2, 80:720], 0)
            nc.gpsimd.memset(u0[:, 2:4, 80:720], 0)

            # chunk-0 critical inputs first; everything later is serialized
            # behind the scatters on the SP FIFO so it cannot steal DMA
            # bandwidth from chunk 0
            nc.scalar.dma_start(
                out=refsb[:, 0 : 22 * C], in_=refv[:, 0 : 22 * C]
            )

            def qb_src(ci, g, sa, sb):
                # [32 part, sb-sa subregions, WINS] strided view of qb
                base = (ci * 4 + g) * 32 * NSUB * WINS
                full = qb[base : base + 32 * NSUB * WINS].rearrange(
                    "(p s j) -> p s j", p=32, j=WINS
                )
                return full[:, sa:sb, :]

            # phase A: subregions 0:4 of each group window
            nc.sync.dma_start(out=win_dst(s0, 0, 0, 4), in_=qb_src(0, 0, 0, 4))
            nc.sync.dma_start(out=win_dst(s0, 1, 0, 4), in_=qb_src(0, 1, 0, 4))
            nc.scalar.dma_start(out=win_dst(s0, 2, 0, 4), in_=qb_src(0, 2, 0, 4))
            nc.gpsimd.dma_start(out=win_dst(s0, 3, 0, 4), in_=qb_src(0, 3, 0, 4))
            # phase B memsets then windows
            nc.vector.memset(u0[:, 4:6, 80:720], 0)
            nc.gpsimd.memset(u0[:, 6:8, 80:720], 0)
            nc.sync.dma_start(out=win_dst(s0, 0, 4, 8), in_=qb_src(0, 0, 4, 8))
            nc.sync.dma_start(out=win_dst(s0, 1, 4, 8), in_=qb_src(0, 1, 4, 8))
            nc.scalar.dma_start(out=win_dst(s0, 2, 4, 8), in_=qb_src(0, 2, 4, 8))
            nc.gpsimd.dma_start(out=win_dst(s0, 3, 4, 8), in_=qb_src(0, 3, 4, 8))
            # first cur rows for chunk-0 evictions (Act, behind its windows)
            nc.scalar.dma_start(
                out=cursb[:, 0 : 16 * C], in_=curv[:, 0 : 16 * C]
            )

            s1 = slot_tile(1)
            u1 = s1[:].rearrange("p (s x) -> p s x", s=NSUB).bitcast(U32)
            nc.vector.memset(u1[:, 0:4, 80:720], 0)
            nc.gpsimd.memset(u1[:, 4:8, 80:720], 0)
            s2 = slot_tile(2)
            u2 = s2[:].rearrange("p (s x) -> p s x", s=NSUB).bitcast(U32)
            nc.vector.memset(u2[:, 0:2, 80:720], 0)
            nc.gpsimd.memset(u2[:, 2:8, 80:720], 0)
            ring = [s0, s1, s2]

            for ci in range(NCH):
                if ci < RING:
                    slot = ring[ci]
                else:
                    slot = slot_tile(0)  # chunk 3: pool-rotated reuse of slot 0
                if ci > 0:
                    for g in range(4):
                        nc.sync.dma_start(
                            out=win_dst(slot, g, 0, NSUB),
                            in_=qb_src(ci, g, 0, NSUB),
                        )
                    # later ref/cur pieces strictly behind this chunk's
                    # windows on the same SP FIFO
                    a, b = REF_PIECES[ci]
                    nc.sync.dma_start(
                        out=refsb[:, a * C : b * C], in_=refv[:, a * C : b * C]
                    )
                    nc.sync.dma_start(
                        out=cursb[:, ci * 16 * C : (ci + 1) * 16 * C],
                        in_=curv[:, ci * 16 * C : (ci + 1) * 16 * C],
                    )

                view = slot[:].rearrange(
                    "p (s c b r) -> p s c b r", s=NSUB, b=NBAND, r=2
                )
                for rp in range(RCH // 2):
                    ps = psum_pool.tile([W, 2 * C], F32, tag="ps")
                    for sub in range(2):
                        rr = 2 * rp + sub
                        h = ci * RCH + rr
                        for b in range(NBAND):
                            lhsT = view[:, rr // 2, 16:144, b, rr % 2]
                            rhs = refsb[:, (h + b) * C : (h + b + 1) * C]
                            nc.tensor.matmul(
                                ps[:, sub * C : (sub + 1) * C],
                                lhsT,
                                rhs,
                                start=(b == 0),
                                stop=(b == NBAND - 1),
                            )
                    he = ci * RCH + 2 * rp
                    nc.vector.tensor_add(
                        outst[:, he * C : (he + 2) * C],
                        ps[:],
                        cursb[:, he * C : (he + 2) * C],
                    )
                # chunk output eviction to DRAM (bf16); last chunk on the
                # idle SP HWDGE queue for a shorter tail
                oeng = nc.sync if ci == NCH - 1 else nc.gpsimd
                oeng.dma_start(
                    out=out[:, ci * RCH : (ci + 1) * RCH, :].rearrange(
                        "w r c -> w (r c)"
                    ),
                    in_=outst[:, ci * RCH * C : (ci + 1) * RCH * C],
                )

    return nc


_NC = None
LAST_RESULT = None


def _get_nc():
    global _NC
    if _NC is None:
        _NC = _build_bass()
    return _NC


# ---------------------------------------------------------------------------
# Host-side shard prep
# ---------------------------------------------------------------------------


def _prep_core(attn_b, rv_b, cv_b, g0):
    """Build one core's in_map. attn_b/rv_b/cv_b: [H, W, ...] for one batch;
    g0: first output row of the shard."""
    # ref with 2-row halo, transposed to [w, r, c]
    refpad = np.zeros((HALO_R, W, C), np.float32)
    lo_g, hi_g = g0 - 2, g0 + HS + 2
    s0, s1 = max(lo_g, 0), min(hi_g, H)
    refpad[s0 - lo_g : s1 - lo_g] = rv_b[s0:s1]
    refhl = refpad.transpose(1, 0, 2).astype(bfloat16)  # [w, r, c]

    # current term pre-scaled by its attention weight (exact fp32 on host)
    cur = cv_b[g0 : g0 + HS] * attn_b[g0 : g0 + HS, :, 25:26]
    curhl = cur.transpose(1, 0, 2).astype(bfloat16)  # [w, h, c]

    # window-packed band values, 2-row sub-interleave: per (chunk, group,
    # partition, subregion) one contiguous 36-col x 5-band x 2-row window;
    # value for (w', j) at window col (w' mod 32) + j, zeros elsewhere
    A = attn_b[g0 : g0 + HS]  # [HS, W, 26]
    wp = np.arange(W)[:, None]
    jj = np.arange(5)[None, :]
    w = wp + jj - 2  # [w', j] out col
    valid = (w >= 0) & (w < W)
    wc = np.clip(w, 0, W - 1)
    kk = 5 * np.arange(5)[:, None] + 4 - np.arange(5)[None, :]  # [b, j]
    Aw = A[:, wc, :]  # [HS, w', j, 26]
    V = np.take_along_axis(Aw, kk.T[None, None, :, :], axis=3)  # [HS,w',j,b]
    V = V * valid[None, :, :, None]
    V7 = V.reshape(NCH, NSUB, 2, 4, 32, 5, 5)  # [c, s, r2, g, pl, j, b]
    W7 = np.zeros((NCH, 4, 32, NSUB, 36, 5, 2), np.float32)
    pl = np.arange(32)
    for j in range(5):
        rhs = V7[:, :, :, :, pl, j, :]  # [c, s, r2, g, pl, b]
        W7[:, :, pl, :, pl + j, :, :] = rhs.transpose(4, 0, 3, 1, 5, 2)
    return {
        "refhl": refhl,
        "curhl": curhl,
        "qb": W7.reshape(-1).astype(bfloat16),
    }


def kernel(attn, ref_value, current_ref_value):
    attn = np.asarray(attn, dtype=np.float32)
    rv = np.asarray(ref_value, dtype=np.float32)
    cv = np.asarray(current_ref_value, dtype=np.float32)

    nc = _get_nc()
    in_maps = []
    for core in range(NCORES):
        bb, half = divmod(core, 2)
        in_maps.append(_prep_core(attn[bb], rv[bb], cv[bb], half * HS))

    res = bass_utils.run_bass_kernel_spmd(nc, in_maps, core_ids=list(range(NCORES)))
    global LAST_RESULT
    LAST_RESULT = res

    out = np.empty((B, H, W, C), np.float32)
    for core in range(NCORES):
        bb, half = divmod(core, 2)
        dev = np.asarray(res.results[core]["out"]).astype(np.float32)  # [w, hs, c]
        out[bb, half * HS : (half + 1) * HS] = dev.transpose(1, 0, 2)
    return out
